# revision 29
# baseline (speedup 1.0000x reference)
"""Trainium2 Bass kernel for nn_AttentionBlock (GroupNorm + MHA + proj + residual).

Sharding: data-parallel over batch (16 batches -> 2 per core x 8 cores).
Weights replicated. Each core computes its 2 batches fully; host gathers.

Per-batch dataflow on a core (c=512, t=1024, H=8 heads, dh=64, 32 groups):
  x [512,1024] -> GroupNorm -> xn
  qk = Wqk_reordered @ xn   (8 o-tiles; pair-ordered so head-pairs share tiles)
  vT = xn^T @ Wv^T          (v produced transposed: [s, c_v], ones col per head)
  per head-pair: logitsT[s,t] = k^T q (row-packed 2 heads in PE array)
                 wT = exp(logitsT)  (ScalarE, PSUM->SBUF)
                 attnRaw[c',t] = vAugT^T @ wT  (c'=65 incl denominator row)
  attn = attnRaw / denom;  out = w_proj @ attn + b_proj + x
"""

import os
import sys

os.environ.setdefault("MYCRO_LOCAL_CACHE", "1")
for _p in ("/root/.axon_site", "/root/.axon_site/_ro/trn_rl_repo",
           "/root/.axon_site/_ro/pypackages", "/opt/trn_rl_repo"):
    if os.path.isdir(_p) and _p not in sys.path:
        sys.path.append(_p)

import numpy as np

from concourse import bass, bacc, tile, mybir
from concourse._compat import get_trn_type
from concourse.bass_utils import run_bass_kernel_spmd

F32 = mybir.dt.float32
F32R = mybir.dt.float32r
BF16 = mybir.dt.bfloat16

N_CORES = 8
B, C, HH, WW = 16, 512, 32, 32
T = HH * WW            # 1024
NHEADS = 8
DH = C // NHEADS       # 64
NGROUPS = 32
GSIZE = C // NGROUPS   # 16 channels per group
EPS = 1e-5
BPC = B // N_CORES     # batches per core = 2
P = 128
NPAIR = NHEADS // 2    # 4 head pairs
CT = C // P            # 4 channel tiles
OT = (2 * C) // P      # 8 qk output tiles
ST = T // P            # 8 s-tiles
TH = T // 512          # 2 t-halves

LAST_RESULTS = None


def _bc_ap(ap, nparts):
    """Broadcast an AP along a new leading partition dim of size nparts."""
    return bass.AP(tensor=ap.tensor, offset=ap.offset,
                   ap=[[0, nparts]] + [list(d) for d in ap.ap])


def build_nc():
    nc = bacc.Bacc(get_trn_type() or "TRN2", target_bir_lowering=False,
                   debug=False)

    xs_d = nc.dram_tensor("xs", [BPC, C, T], F32, kind="ExternalInput")
    wqkT_d = nc.dram_tensor("wqkT", [C, 2 * C], F32R, kind="ExternalInput")
    bqkT_d = nc.dram_tensor("bqkT", [P, OT], F32, kind="ExternalInput")
    wvT_d = nc.dram_tensor("wvT", [C, C], F32R, kind="ExternalInput")
    bvA_d = nc.dram_tensor("bvA", [NHEADS * 65], F32, kind="ExternalInput")
    wpT_d = nc.dram_tensor("wpT", [C, C], F32R, kind="ExternalInput")
    bpT_d = nc.dram_tensor("bpT", [P, CT], F32, kind="ExternalInput")
    gnT_d = nc.dram_tensor("gnT", [2, C], F32, kind="ExternalInput")
    out_d = nc.dram_tensor("out", [BPC, C, T], F32, kind="ExternalOutput")

    from contextlib import ExitStack
    with ExitStack() as ctx:
        tc = ctx.enter_context(tile.TileContext(nc))
        cpool = ctx.enter_context(tc.tile_pool(name="const", bufs=1))
        xpool = ctx.enter_context(tc.tile_pool(name="xp", bufs=5))
        xnpool = ctx.enter_context(tc.tile_pool(name="xnp", bufs=4))
        qkpool = ctx.enter_context(tc.tile_pool(name="qkp", bufs=8))
        vtpool = ctx.enter_context(tc.tile_pool(name="vtp", bufs=9))
        wtpool = ctx.enter_context(tc.tile_pool(name="wtp", bufs=3))
        accpool = ctx.enter_context(tc.tile_pool(name="accp", bufs=3))
        attnpool = ctx.enter_context(tc.tile_pool(name="attnp", bufs=4))
        outpool = ctx.enter_context(tc.tile_pool(name="outp", bufs=2))
        smallpool = ctx.enter_context(tc.tile_pool(name="smallp", bufs=2))
        medpool = ctx.enter_context(tc.tile_pool(name="medp", bufs=2))
        scrpool = ctx.enter_context(tc.tile_pool(name="scrp", bufs=1))
        dramp = ctx.enter_context(tc.tile_pool(name="dramp", bufs=2, space="DRAM"))
        ps_mm = ctx.enter_context(tc.tile_pool(name="ps_mm", bufs=2, space="PSUM"))
        ps_lg = ctx.enter_context(tc.tile_pool(name="ps_lg", bufs=2, space="PSUM"))
        ps_at = ctx.enter_context(tc.tile_pool(name="ps_at", bufs=2, space="PSUM"))
        if True:
            dma = nc.default_dma_engine

            # ---- one-time constant loads ----
            wqkT_sb = []
            for k in range(CT):
                w = cpool.tile([P, 2 * C], F32R, tag=f"wqkT{k}")
                dma.dma_start(w[:], wqkT_d[P * k:P * (k + 1), :])
                wqkT_sb.append(w)
            wvT_sb = []
            for k in range(CT):
                w = cpool.tile([P, C], F32R, tag=f"wvT{k}")
                dma.dma_start(w[:], wvT_d[P * k:P * (k + 1), :])
                wvT_sb.append(w)
            wpT_sb = []
            for k in range(CT):
                w = cpool.tile([P, C], F32R, tag=f"wpT{k}")
                dma.dma_start(w[:], wpT_d[P * k:P * (k + 1), :])
                wpT_sb.append(w)
            bqk_sb = cpool.tile([P, OT], F32, tag="bqk")
            dma.dma_start(bqk_sb[:], bqkT_d[:])
            bp_sb = cpool.tile([P, CT], F32, tag="bp")
            dma.dma_start(bp_sb[:], bpT_d[:])
            gn_s_sb = cpool.tile([1, C], F32, tag="gns")
            dma.dma_start(gn_s_sb[:], gnT_d[0:1, :])
            gn_b_sb = cpool.tile([1, C], F32, tag="gnb")
            dma.dma_start(gn_b_sb[:], gnT_d[1:2, :])
            bv_bc = cpool.tile([P, NHEADS * 65], F32, tag="bv")
            dma.dma_start(bv_bc[:], _bc_ap(bvA_d.ap(), P))
            zero_b = cpool.tile([P, 1], F32, tag="zerob")
            nc.vector.memset(zero_b[:], 0.0)
            eps_b = cpool.tile([1, 1], F32, tag="epsb")
            nc.vector.memset(eps_b[:], EPS)

            for b in range(BPC):
                # ---- load x ----
                x_sb = []
                for j in range(CT):
                    xt = xpool.tile([P, T], F32, tag="x")
                    dma.dma_start(xt[:], xs_d[b, P * j:P * (j + 1), :])
                    x_sb.append(xt)

                # ---- GroupNorm stats: per-row mean/var then group combine ----
                st_dram = dramp.tile([C, 3], F32, tag="stats")
                s3 = smallpool.tile([P, CT, 3], F32, tag="s3")
                for j in range(CT):
                    bst = smallpool.tile([P, 2, 6], F32, tag="bst")
                    for sg in range(2):
                        nc.vector.bn_stats(out=bst[:, sg, :],
                                           in_=x_sb[j][:, 512 * sg:512 * (sg + 1)])
                    mv = smallpool.tile([P, 2], F32, tag="mv")
                    nc.vector.bn_aggr(out=mv[:], in_=bst[:])
                    nc.vector.tensor_copy(s3[:, j, 0:2], mv[:])
                    nc.vector.tensor_mul(s3[:, j, 2:3], mv[:, 0:1], mv[:, 0:1])
                # one DMA: (p, j, r) -> dram[(128j + p)*3 + r]
                st_ap0 = st_dram[:]
                dst = bass.AP(tensor=st_ap0.tensor, offset=st_ap0.offset,
                              ap=[[3, P], [3 * P, CT], [1, 3]])
                dma.dma_start(dst, s3[:])

                # read back transposed, one [1, C] row tile per stat
                st_ap = st_dram[:]
                strow = []
                for r in range(3):
                    rt = smallpool.tile([1, C], F32, tag=f"strow{r}")
                    src = bass.AP(tensor=st_ap.tensor, offset=st_ap.offset + r,
                                  ap=[[3, 1], [3, C]])
                    dma.dma_start(rt[:], src)
                    strow.append(rt)

                # group-sum each: [1, 32, 16] -> [1, 32]
                stG = []
                for r in range(3):
                    g = smallpool.tile([1, NGROUPS], F32, tag=f"stG{r}")
                    nc.vector.tensor_reduce(
                        out=g[:], in_=strow[r][:].rearrange(
                            "p (g d) -> p g d", d=GSIZE),
                        axis=mybir.AxisListType.X, op=mybir.AluOpType.add)
                    stG.append(g)

                # grp[1, 64] = [mu(32) | rstd(32)]  (on partition 0)
                grp = smallpool.tile([1, 2 * NGROUPS], F32, tag="grp")
                mu = grp[:, 0:NGROUPS]
                nc.vector.tensor_scalar_mul(mu, stG[0][:], 1.0 / GSIZE)
                tmp = smallpool.tile([1, NGROUPS], F32, tag="gtmp")
                nc.vector.tensor_add(tmp[:], stG[1][:], stG[2][:])
                musq = smallpool.tile([1, NGROUPS], F32, tag="gmusq")
                nc.vector.tensor_mul(musq[:], mu, mu)
                var = smallpool.tile([1, NGROUPS], F32, tag="gvar")
                nc.vector.scalar_tensor_tensor(
                    out=var[:], in0=tmp[:], scalar=1.0 / GSIZE, in1=musq[:],
                    op0=mybir.AluOpType.mult, op1=mybir.AluOpType.subtract)
                # rstd = exp(-0.5 * ln(var + eps)) -- stays in exp/ln table set
                lnv = smallpool.tile([1, NGROUPS], F32, tag="glnv")
                nc.scalar.activation(lnv[:], var[:],
                                     mybir.ActivationFunctionType.Ln,
                                     bias=eps_b[:])
                nc.scalar.activation(grp[:, NGROUPS:2 * NGROUPS], lnv[:],
                                     mybir.ActivationFunctionType.Exp,
                                     bias=zero_b[0:1, :], scale=-0.5)

                # per-channel affine in transposed domain:
                #   a[c] = rstd_g(c) * scale[c];  b[c] = bias[c] - mu_g(c) * a[c]
                def _rep16(sl):
                    return bass.AP(tensor=sl.tensor, offset=sl.offset,
                                   ap=[list(sl.ap[0]), [1, NGROUPS], [0, GSIZE]])

                abT = smallpool.tile([1, 2 * C], F32, tag="abT")
                abT_a = abT[:, 0:C]
                abT_b = abT[:, C:2 * C]
                a3 = abT_a.rearrange("p (g d) -> p g d", d=GSIZE)
                nc.vector.tensor_mul(a3, _rep16(grp[:, NGROUPS:2 * NGROUPS]),
                                     gn_s_sb[:].rearrange("p (g d) -> p g d",
                                                          d=GSIZE))
                mua = smallpool.tile([1, C], F32, tag="mua")
                nc.vector.tensor_mul(mua[:].rearrange("p (g d) -> p g d", d=GSIZE),
                                     _rep16(grp[:, 0:NGROUPS]), a3)
                nc.vector.tensor_sub(abT_b, gn_b_sb[:], mua[:])
                ab_dram = dramp.tile([2, C], F32, tag="abd")
                dma.dma_start(ab_dram[:], abT[:])

                # ---- apply GN per tile ----
                ab_ap = ab_dram[:]
                xn_sb = []
                for j in range(CT):
                    abj = smallpool.tile([P, 2], F32, tag="abj")
                    src = bass.AP(tensor=ab_ap.tensor,
                                  offset=ab_ap.offset + P * j,
                                  ap=[[1, P], [C, 2]])
                    dma.dma_start(abj[:], src)
                    xn = xnpool.tile([P, T], F32R, tag="xn")
                    nc.vector.tensor_scalar(
                        out=xn[:], in0=x_sb[j][:], scalar1=abj[:, 0:1],
                        scalar2=abj[:, 1:2],
                        op0=mybir.AluOpType.mult, op1=mybir.AluOpType.add)
                    xn_sb.append(xn)

                # ---- QK matmul: qk[j][p, t] bf16, j even = q-pair, odd = k-pair
                qk_sb = []
                for j in range(OT):
                    qk = qkpool.tile([P, T], BF16, tag="qk")
                    for th in range(TH):
                        ps = ps_mm.tile([P, 512], F32, tag="psmm")
                        for k in range(CT):
                            nc.tensor.matmul(
                                ps[:],
                                wqkT_sb[k][:, P * j:P * (j + 1)],
                                xn_sb[k][:, 512 * th:512 * (th + 1)],
                                start=(k == 0), stop=(k == CT - 1))
                        nc.vector.tensor_scalar_add(
                            qk[:, 512 * th:512 * (th + 1)], ps[:],
                            bqk_sb[:, j:j + 1])
                    qk_sb.append(qk)

                # ---- vT matmul: vt[st][s_local, 8*65] with ones cols ----
                vt_sb = []
                for st in range(ST):
                    vt = vtpool.tile([P, NHEADS * 65], F32R, tag="vt")
                    vt3 = vt[:].rearrange("p (h c) -> p h c", h=NHEADS)
                    ps = ps_mm.tile([P, 512], F32, tag="psmm")
                    for k in range(CT):
                        nc.tensor.matmul(
                            ps[:],
                            xn_sb[k][:, P * st:P * (st + 1)],
                            wvT_sb[k][:],
                            start=(k == 0), stop=(k == CT - 1))
                    bv3 = bv_bc[:].rearrange("p (h c) -> p h c", h=NHEADS)
                    nc.vector.tensor_add(
                        vt3[:, :, 0:DH],
                        ps[:].rearrange("p (h c) -> p h c", h=NHEADS),
                        bv3[:, :, 0:DH])
                    nc.vector.tensor_copy(vt3[:, :, DH:DH + 1], bv3[:, :, DH:DH + 1])
                    vt_sb.append(vt)

                # ---- attention per head pair ----
                den = scrpool.tile([NHEADS, T], F32, tag="den")
                attn_sb = []
                for p_i in range(NPAIR):
                    qt = qk_sb[2 * p_i]
                    kt = qk_sb[2 * p_i + 1]
                    accA = accpool.tile([65, T], F32, tag="acc")
                    accB = accpool.tile([65, T], F32, tag="acc")
                    for st in range(ST):
                        wts = []
                        for hh in range(2):
                            lg = ps_lg.tile([P, T], F32, tag="pslg")
                            lo = 64 * hh
                            for th in range(TH):
                                nc.tensor.matmul(
                                    lg[:, 512 * th:512 * (th + 1)],
                                    kt[lo:lo + DH, P * st:P * (st + 1)],
                                    qt[lo:lo + DH, 512 * th:512 * (th + 1)],
                                    start=True, stop=True)
                            wt = wtpool.tile([P, T], F32R, tag="wt")
                            nc.scalar.activation(wt[:], lg[:],
                                                 mybir.ActivationFunctionType.Exp,
                                                 bias=zero_b[:])
                            wts.append(wt)
                        for hh, acc in ((0, accA), (1, accB)):
                            h_abs = 2 * p_i + hh
                            vslice = vt_sb[st][:, 65 * h_abs:65 * (h_abs + 1)]
                            for th in range(TH):
                                pa = ps_at.tile([65, 512], F32, tag="psat")
                                nc.tensor.matmul(
                                    pa[:], vslice,
                                    wts[hh][:, 512 * th:512 * (th + 1)],
                                    start=True, stop=True)
                                dst = acc[:, 512 * th:512 * (th + 1)]
                                if st == 0:
                                    nc.vector.tensor_copy(dst, pa[:])
                                else:
                                    nc.vector.tensor_add(dst, dst, pa[:])
                    # stage un-normalized attn; stash denominators; free accs
                    at = attnpool.tile([P, T], F32R, tag="attn")
                    for hh, acc in ((0, accA), (1, accB)):
                        h_abs = 2 * p_i + hh
                        nc.vector.tensor_copy(at[64 * hh:64 * hh + DH, :],
                                              acc[0:DH, :])
                        dma.dma_start(den[h_abs:h_abs + 1, :], acc[DH:DH + 1, :])
                    attn_sb.append(at)

                # ---- softmax denominators -> reciprocal ----
                rec = scrpool.tile([NHEADS, T], F32, tag="rec")
                scr = scrpool.tile([NHEADS, T], F32, tag="scr")
                nc.vector.reciprocal_approx_accurate(out=rec[:], in_=den[:],
                                                     scratch=scr[:])
                rec_dram = dramp.tile([NHEADS, T], F32, tag="recd")
                dma.dma_start(rec_dram[:], rec[:])

                # ---- divide in place ----
                rd_ap = rec_dram[:]
                for p_i in range(NPAIR):
                    rbc = medpool.tile([P, T], F32, tag="rbc")
                    for hh in range(2):
                        h_abs = 2 * p_i + hh
                        src = bass.AP(tensor=rd_ap.tensor,
                                      offset=rd_ap.offset + T * h_abs,
                                      ap=[[0, DH], [1, T]])
                        dma.dma_start(rbc[64 * hh:64 * hh + DH, :], src)
                    nc.vector.tensor_mul(attn_sb[p_i][:], attn_sb[p_i][:],
                                         rbc[:])

                # ---- proj + bias + residual ----
                for j in range(CT):
                    ot = outpool.tile([P, T], F32, tag="out")
                    for th in range(TH):
                        ps = ps_mm.tile([P, 512], F32, tag="psmm")
                        for k in range(CT):
                            nc.tensor.matmul(
                                ps[:],
                                wpT_sb[k][:, P * j:P * (j + 1)],
                                attn_sb[k][:, 512 * th:512 * (th + 1)],
                                start=(k == 0), stop=(k == CT - 1))
                        nc.vector.scalar_tensor_tensor(
                            out=ot[:, 512 * th:512 * (th + 1)], in0=ps[:],
                            scalar=bp_sb[:, j:j + 1],
                            in1=x_sb[j][:, 512 * th:512 * (th + 1)],
                            op0=mybir.AluOpType.add, op1=mybir.AluOpType.add)
                    dma.dma_start(out_d[b, P * j:P * (j + 1), :], ot[:])

    nc.compile()
    return nc


def prep_inputs(x, gn_scale, gn_bias, w_qkv, b_qkv, w_proj, b_proj):
    """Host-side: reorder/scale weights, build per-core input maps."""
    x2 = np.ascontiguousarray(
        np.asarray(x, dtype=np.float32).reshape(B, C, T))
    w_qkv = np.asarray(w_qkv, dtype=np.float32)
    b_qkv = np.asarray(b_qkv, dtype=np.float32)
    scale = float(DH) ** -0.25

    qk_rows = []
    for p_i in range(NPAIR):
        for hh in range(2):           # q rows of the pair
            h = 2 * p_i + hh
            qk_rows.extend(range(192 * h, 192 * h + DH))
        for hh in range(2):           # k rows of the pair
            h = 2 * p_i + hh
            qk_rows.extend(range(192 * h + DH, 192 * h + 2 * DH))
    qk_rows = np.array(qk_rows)
    wqkT = np.ascontiguousarray((w_qkv[qk_rows] * scale).T)
    bqkT = np.ascontiguousarray(
        (b_qkv[qk_rows] * scale).reshape(OT, P).T)

    v_rows = np.array([192 * h + 2 * DH + j for h in range(NHEADS)
                       for j in range(DH)])
    wvT = np.ascontiguousarray(w_qkv[v_rows].T)
    bv = b_qkv[v_rows]
    bvA = np.zeros(NHEADS * 65, np.float32)
    for h in range(NHEADS):
        bvA[65 * h:65 * h + DH] = bv[DH * h:DH * (h + 1)]
        bvA[65 * h + DH] = 1.0

    wpT = np.ascontiguousarray(np.asarray(w_proj, np.float32).T)
    bpT = np.ascontiguousarray(
        np.asarray(b_proj, np.float32).reshape(CT, P).T)
    gnT = np.ascontiguousarray(np.stack(
        [np.asarray(gn_scale, np.float32),
         np.asarray(gn_bias, np.float32)], axis=0))

    common = dict(wqkT=wqkT, bqkT=bqkT, wvT=wvT, bvA=bvA, wpT=wpT,
                  bpT=bpT, gnT=gnT)
    in_maps = [dict(common, xs=np.ascontiguousarray(x2[BPC * i:BPC * (i + 1)]))
               for i in range(N_CORES)]
    return in_maps


_NC = None


def _ensure_ntff_hook():
    """The agent image's antenv lacks axon_hooks; shim it and register the
    ctypes NTFF hook from the boot script so trace=True can measure HW time."""
    try:
        from antenv import axon_hooks  # noqa: F401
        return
    except ImportError:
        pass
    import types
    import antenv
    mod = types.ModuleType("antenv.axon_hooks")
    _state = {"fn": None}
    mod.set_axon_ntff_profile_hook = lambda fn: _state.__setitem__("fn", fn)
    mod.get_axon_ntff_profile_hook = lambda: _state["fn"]
    sys.modules["antenv.axon_hooks"] = mod
    antenv.axon_hooks = mod
    try:
        from trn_agent_boot.trn_boot import _ntff_profile_via_ctypes
        hook = _ntff_profile_via_ctypes("/opt/axon/libaxon_pjrt.so")
        mod.set_axon_ntff_profile_hook(hook)
    except Exception as e:  # degrade: run proceeds untraced
        print("ntff hook setup failed:", e)


def kernel(x, gn_scale, gn_bias, w_qkv, b_qkv, w_proj, b_proj):
    global _NC, LAST_RESULTS
    if _NC is None:
        _NC = build_nc()
    in_maps = prep_inputs(x, gn_scale, gn_bias, w_qkv, b_qkv, w_proj, b_proj)
    trace = bool(os.environ.get("KERNEL_TRACE"))
    if trace:
        _ensure_ntff_hook()
    res = run_bass_kernel_spmd(_NC, in_maps, list(range(N_CORES)), trace=trace)
    LAST_RESULTS = res
    out = np.concatenate([res.results[i]["out"] for i in range(N_CORES)],
                         axis=0)
    return out.reshape(B, C, HH, WW).astype(np.float32)
